# revision 27
# baseline (speedup 1.0000x reference)
"""Energy-model kernel for Trainium2, data-parallel over 8 NeuronCores.

E[b] = 0.5||x||^2 + 0.5||z||^2 - (phi_vis + phi_enc + phi_bias + phi_pos
       + phi_mem + phi_att)

Host staging (pure data movement, bf16): im2col view of x (the stride-8
conv is a patch matmul), z both row-major and pre-transposed (ztr), all
weights pre-arranged for lhsT use, and the combined z|x bias constant.

Per core: 128 samples as 64 blocks of 2, pairs of blocks share one zT
load and one Q/K matmul group, 8 blocks per batched vector stage:
  - zT (d, (blk,s,p)) bf16 arrives by plain DMA from ztr; all matmuls
    (y=z@Wenc, mem=z@Mw, Q/K, A) run in bf16 with fp32 PSUM.
  - A = Q^T K per (sample, head) is one k=128 matmul per (s, headgroup)
    against a block-diagonal K built by gpsimd as broadcast*diag-mask
    (the runtime rejects concurrent k=32 row-tiled matmuls).
  - logsumexp without max-subtraction (|gamma*A| < ~6): one Exp per
    block, batched reduce, Ln, head-sum.
  - quad/bias terms use 0.5*(v-b)^2; the constant 0.5 b^2 is corrected
    on the host. z and x share one tile so sub+Square-accum fuse.
  - memory term: DVE relu (ACT for every other block's 2nd chunk for
    balance), ACT Square with free-axis accumulate.
  - per-(partition, block) partials accumulate in accA planes
    [zx-quad, enc, mem, lse]; the final cross-partition sum (512 values
    per sample) happens on the host during unsharding.

Known-good engine balance at ~238us/core: DVE ~190us, ACT ~190us,
PE ~153us, GpSimd ~121us. walrus here accepts only one sync wait per
instruction -> _split_excess_waits hoists extras onto nop carriers.
"""
import sys
import types

sys.path.insert(0, "/opt/trn_rl_repo")

import numpy as np
import ml_dtypes

import concourse.bass as bass
import concourse.mybir as mybir
import concourse.tile as tile_mod
import bass_rust
from concourse.tile import TileContext
from concourse.bass_utils import run_bass_kernel_spmd

# ---------------------------------------------------------------- shims
def _split_excess_waits(nc):
    """walrus in this env accepts a single sync wait per instruction, but
    Tile attaches several. Hoist extras onto nop carriers on the same
    engine, placed just before the instruction (engine program order)."""
    cnt = 0
    for f in nc.m.functions:
        for blk in f.blocks:
            il = blk.instructions
            new = []
            for inst in il:
                si = inst.sync_info
                waits = list(si.on_wait or []) if si is not None else []
                if len(waits) > 1:
                    for w in waits[1:]:
                        nop = mybir.InstNoOp(name=f"WSPLIT-{cnt}", ins=[], outs=[])
                        cnt += 1
                        nop.engine = inst.engine
                        nop.sync_info = mybir.SyncInfo(on_wait=[w], on_update=[])
                        new.append(nop)
                    inst.sync_info = mybir.SyncInfo(
                        on_wait=[waits[0]], on_update=list(si.on_update or [])
                    )
                new.append(inst)
            if len(new) != len(il):
                il.clear()
                il.extend(new)
    return cnt


def _install_ntff_hook():
    if "antenv.axon_hooks" in sys.modules:
        return
    mod = types.ModuleType("antenv.axon_hooks")
    state = {"hook": None}
    mod.set_axon_ntff_profile_hook = lambda h: state.__setitem__("hook", h)
    mod.get_axon_ntff_profile_hook = lambda: state["hook"]
    sys.modules["antenv.axon_hooks"] = mod
    try:
        import antenv

        antenv.axon_hooks = mod
        from trn_agent_boot.trn_boot import _ntff_profile_via_ctypes

        mod.set_axon_ntff_profile_hook(
            _ntff_profile_via_ctypes("/opt/axon/libaxon_pjrt.so")
        )
    except Exception:
        pass


_install_ntff_hook()

# ---------------------------------------------------------------- consts
N_CORES = 8
B, C, H = 1024, 3, 64
D, NP, M, NH, R, P = 256, 64, 1024, 8, 32, 8
GAMMA = 0.25
BC = B // N_CORES          # samples per core
NB = BC // 2               # blocks of 2 samples
KCPP = C * P * P           # 192 patch elements
NT = 4                     # partial planes: zx-quad, enc, mem, lse
F32 = mybir.dt.float32
BF16 = mybir.dt.bfloat16
FP8 = mybir.dt.float8e4


def _build_nc(trace_scope=False, nb=NB):
    G = 8                      # blocks per batched vector stage
    if nb < G:
        G = nb
    assert nb % G == 0
    g_count = nb // G
    assert G % 2 == 0 or nb == 1
    nc = bass.Bass()
    x_d = nc.dram_tensor("x", [BC // 2, 128, KCPP], BF16, kind="ExternalInput")
    z_d = nc.dram_tensor("z", [BC // 2, 128, D], BF16, kind="ExternalInput")
    ztr_d = nc.dram_tensor("ztr", [BC, D, NP], BF16, kind="ExternalInput")
    mw_d = nc.dram_tensor("mw", [D, M], BF16, kind="ExternalInput")
    wqk_d = nc.dram_tensor("wqk", [D, 2 * NH * R], BF16, kind="ExternalInput")
    wenc_d = nc.dram_tensor("wenc", [D, KCPP], BF16, kind="ExternalInput")
    dmask_d = nc.dram_tensor("dmask", [128, 4], BF16, kind="ExternalInput")
    nmask_d = nc.dram_tensor("nmask", [128, 2], BF16, kind="ExternalInput")
    ident_d = nc.dram_tensor("ident", [128, 128], BF16, kind="ExternalInput")
    out_d = nc.dram_tensor("out", [128, NT * nb], F32, kind="ExternalOutput")

    with TileContext(nc) as tc:
        import contextlib

        with contextlib.ExitStack() as ctx:
            singles = ctx.enter_context(tc.tile_pool(name="singles", bufs=1))
            gpool = ctx.enter_context(tc.tile_pool(name="gpool", bufs=4))
            sbsm = ctx.enter_context(tc.tile_pool(name="sbsm", bufs=5))
            scr = ctx.enter_context(tc.tile_pool(name="scr", bufs=4))
            psQK = ctx.enter_context(tc.tile_pool(name="psQK", bufs=1, space="PSUM"))
            psA = ctx.enter_context(tc.tile_pool(name="psA", bufs=1, space="PSUM"))
            psMem = ctx.enter_context(tc.tile_pool(name="psMem", bufs=2, space="PSUM"))
            psY = ctx.enter_context(tc.tile_pool(name="psY", bufs=1, space="PSUM"))
            psGram = ctx.enter_context(
                tc.tile_pool(name="psGram", bufs=1, space="PSUM")
            )
            psPark = ctx.enter_context(
                tc.tile_pool(name="psPark", bufs=1, space="PSUM")
            )

            # constants
            mw_sb = singles.tile([128, 2, M], BF16)
            nc.sync.dma_start(out=mw_sb, in_=mw_d.rearrange("(k p) m -> p k m", p=128))
            wqk_sb = singles.tile([128, 2, 2 * NH * R], BF16)
            nc.sync.dma_start(
                out=wqk_sb, in_=wqk_d.rearrange("(k p) m -> p k m", p=128)
            )
            wenc_sb = singles.tile([128, 2, KCPP], BF16)
            nc.sync.dma_start(
                out=wenc_sb, in_=wenc_d.rearrange("(k p) m -> p k m", p=128)
            )

            dmask_sb = singles.tile([128, 4], BF16)
            nc.sync.dma_start(out=dmask_sb, in_=dmask_d[:, :])
            nmask_sb = singles.tile([128, 2], BF16)
            nc.sync.dma_start(out=nmask_sb, in_=nmask_d[:, :])
            ident_sb = singles.tile([128, 128], BF16)
            nc.sync.dma_start(out=ident_sb, in_=ident_d[:, :])

            accA = singles.tile([128, NT, nb], F32)
            # parked attention exp-sums: col = 8*j0 + 8*bi + 2*gu + h
            park = psPark.tile([128, 512], F32)
            # persistent block-diag K buffers (off-diag zeros live across
            # pairs for the DMA-built half)
            bkds = [singles.tile([128, 2, 4, 4, 64], BF16, name=f"bkd{i}")
                    for i in range(2)]
            for t_ in bkds:
                nc.vector.memset(t_, 0.0)

            for jg in range(g_count):
                zxbig = gpool.tile([128, G, D + KCPP], BF16, tag="zxbig")
                ybuf = gpool.tile([128, G, KCPP], BF16, tag="ybuf")

                nc.sync.dma_start(
                    out=zxbig[:, :, :D].rearrange("q g d -> q g d"),
                    in_=z_d[jg * G : (jg + 1) * G].rearrange("j q d -> q j d"),
                )
                nc.sync.dma_start(
                    out=zxbig[:, :, D:].rearrange("q g k -> q g k"),
                    in_=x_d[jg * G : (jg + 1) * G].rearrange("j q k -> q j k"),
                )

                for jp in range(G // 2):
                    # ---- pair of blocks shares zT / qk
                    zT = sbsm.tile([128, 2, 256], BF16, tag="zt")
                    j0 = jg * G + 2 * jp
                    for kc in range(2):
                        # ztr[(4 samples), kc-chunk, :] -> (dp, (blk s p))
                        nc.sync.dma_start(
                            out=zT[:, kc, :].rearrange("d (s p) -> d s p", s=4),
                            in_=ztr_d[
                                2 * j0 : 2 * j0 + 4,
                                128 * kc : 128 * (kc + 1), :,
                            ].rearrange("s d p -> d s p"),
                        )
                    # ---- Q,K for the pair (n = 256)
                    qk_ps = psQK.tile([128, 4, 256], F32, tag="qk")
                    for g in range(4):
                        for kc in range(2):
                            nc.tensor.matmul(
                                qk_ps[:, g, :],
                                wqk_sb[:, kc, 128 * g : 128 * (g + 1)],
                                zT[:, kc, :],
                                start=(kc == 0), stop=(kc == 1),
                            )
                    qk = sbsm.tile([128, 4, 256], BF16, tag="qk_bf")
                    nc.vector.tensor_copy(qk, qk_ps)

                    # ---- block-diagonal K for A^T: per g: [128=(h4,r32),
                    #      (s4, h'4, n64)], nonzero iff h==h'.
                    # g=0 via gpsimd broadcast*mask; g=1 via SBUF->SBUF DMA
                    # diag copies into a pre-zeroed persistent buffer.
                    bkd = bkds[(jg * (G // 2) + jp) % 2]
                    kv = qk[:, 2, :]
                    kb = bass.AP(
                        tensor=kv.tensor, offset=kv.offset,
                        ap=[list(kv.ap[0]), [64, 4], [0, 4], [1, 64]],
                    )
                    dm = dmask_sb[:, :]
                    dmb = bass.AP(
                        tensor=dm.tensor, offset=dm.offset,
                        ap=[list(dm.ap[0]), [0, 4], [1, 4], [0, 64]],
                    )
                    nc.gpsimd.tensor_mul(bkd[:, 0, :, :, :], kb, dmb)
                    for h in range(4):
                        nc.sync.dma_start(
                            out=bkd[32 * h : 32 * h + 32, 1, :, h, :],
                            in_=qk[32 * h : 32 * h + 32, 3, :]
                            .rearrange("p (s n) -> p s n", s=4),
                        )
                    ps_y = psY.tile([128, 2, 256], F32, tag="ypair")

                    for bi in range(2):
                        jj = 2 * jp + bi
                        j = jg * G + jj
                        zTb = zT[:, :, 128 * bi : 128 * (bi + 1)]

                        # ---- y = z @ Wenc ; stash y*xp for batched reduce
                        for kc in range(2):
                            nc.tensor.matmul(
                                ps_y[:, bi, :KCPP], zTb[:, kc, :],
                                wenc_sb[:, kc, :],
                                start=(kc == 0), stop=(kc == 1),
                            )
                        nc.vector.tensor_mul(
                            ybuf[:, jj, :], ps_y[:, bi, :KCPP], zxbig[:, jj, D:]
                        )

                    # ---- memory term, transposed: m on partitions, then
                    # PE Gram rT^T rT accumulated over the 8 m-chunks;
                    # sum(relu^2) per (sample,pos) is the Gram diagonal.
                    rT8 = sbsm.tile([128, 8, 256], FP8, tag="rT8")
                    for t in range(4):
                        mpT = psMem.tile(
                            [128, 2, 256], F32, tag="mem", name=f"mpT{j0}_{t}"
                        )
                        for u in range(2):
                            mc = 2 * t + u
                            for kc in range(2):
                                nc.tensor.matmul(
                                    mpT[:, u, :],
                                    mw_sb[:, kc, 128 * mc : 128 * (mc + 1)],
                                    zT[:, kc, :],
                                    start=(kc == 0), stop=(kc == 1),
                                )
                        dst = rT8[:, 2 * t : 2 * t + 2, :].rearrange(
                            "p c n -> p (c n)")
                        src = mpT.rearrange("p c n -> p (c n)")
                        if t % 2 == 0:
                            nc.scalar.activation(
                                out=dst, in_=src,
                                func=mybir.ActivationFunctionType.Relu,
                            )
                        else:
                            nc.vector.tensor_scalar_max(dst, src, 0.0)

                    # ---- A^T halves per g: 1-bank psum each, exp+nsum per g
                    escT = sbsm.tile([128, 16, 64], BF16, tag="escT")
                    for g in range(2):
                        a_ps = psA.tile([128, 8, 64], F32, tag="a", name=f"a{g}")
                        for s4 in range(4):
                            bi2, sl2 = s4 // 2, s4 % 2
                            for u in range(2):
                                cl = u * 4 + 2 * bi2 + sl2
                                nc.tensor.matmul(
                                    a_ps[:, cl, :],
                                    bkd[:, g, s4, 2 * u : 2 * u + 2, :].rearrange(
                                        "p h n -> p (h n)"
                                    ),
                                    qk[:, g, 64 * s4 : 64 * (s4 + 1)],
                                    start=True, stop=True,
                                )
                        nc.scalar.activation(
                            out=escT[:, 8 * g : 8 * g + 8, :].rearrange(
                                "p c n -> p (c n)"
                            ),
                            in_=a_ps.rearrange("p c n -> p (c n)"),
                            func=mybir.ActivationFunctionType.Exp,
                            scale=GAMMA,
                        )
                        for u in range(2):
                            for bi2 in range(2):
                                gu = 2 * g + u
                                col = 8 * j0 + 8 * bi2 + 2 * gu
                                nc.tensor.matmul(
                                    park[:, col : col + 2],
                                    escT[:, 8 * g + 4 * u + 2 * bi2 :
                                         8 * g + 4 * u + 2 * bi2 + 2, :]
                                    .rearrange("p c n -> p (c n)"),
                                    nmask_sb,
                                    start=True, stop=True,
                                )

                    # ---- mem Gram: accumulate rT^T rT over m-chunks, then
                    # pull the diagonal (per-column sum relu^2) with one
                    # fused STT against identity per block
                    gram = psGram.tile([128, 2, 128], F32, tag="gram",
                                       name=f"g{j0}")
                    for hh in range(2):
                        for mc in range(8):
                            nc.tensor.matmul(
                                gram[:, hh, :],
                                rT8[:, mc, 128 * hh : 128 * (hh + 1)],
                                rT8[:, mc, 128 * hh : 128 * (hh + 1)],
                                start=(mc == 0), stop=(mc == 7),
                            )
                    for hh in range(2):
                        dex = scr.tile([128, 128], BF16, tag="dex")
                        nc.vector.scalar_tensor_tensor(
                            out=dex, in0=gram[:, hh, :], scalar=1.0,
                            in1=ident_sb,
                            op0=mybir.AluOpType.mult, op1=mybir.AluOpType.mult,
                            accum_out=accA[:, 2, j0 + hh : j0 + hh + 1],
                        )

                # ---- batched vector stages, half-super granularity
                # (bias already subtracted on the host: zxbig holds v-b)
                Gh = G // 2
                for hb in range(2):
                    sl = slice(hb * Gh, (hb + 1) * Gh)
                    jsl = slice(jg * G + hb * Gh, jg * G + (hb + 1) * Gh)
                    for jj in range(hb * Gh, (hb + 1) * Gh):
                        j = jg * G + jj
                        nc.scalar.activation(
                            out=zxbig[:, jj, :], in_=zxbig[:, jj, :],
                            func=mybir.ActivationFunctionType.Square,
                            accum_out=accA[:, 0, j : j + 1],
                        )
                    nc.vector.tensor_reduce(
                        out=accA[:, 1, jsl], in_=ybuf[:, sl, :, :]
                        if len(ybuf.shape) == 4 else ybuf[:, sl, :],
                        axis=mybir.AxisListType.X, op=mybir.AluOpType.add,
                    )

            # ---- one batched Ln over all parked exp-sums, one reduce
            lns_all = singles.tile([128, 512], F32)
            nc.scalar.activation(
                out=lns_all, in_=park,
                func=mybir.ActivationFunctionType.Ln,
            )
            nc.vector.tensor_reduce(
                out=accA[:, 3, :],
                in_=lns_all.rearrange("p (b k) -> p b k", k=8),
                axis=mybir.AxisListType.X, op=mybir.AluOpType.add,
            )

            # ---- ship per-partition partials; host does the tiny
            # cross-partition reduction as part of unsharding
            nc.sync.dma_start(
                out=out_d[:, :], in_=accA.rearrange("p t j -> p (t j)")
            )

    _split_excess_waits(nc)
    return nc


_CACHE = {}


def kernel(x, z, encoder_weight, encoder_bias, visible_bias, pos_bias,
           memory_weight, W_Q, W_K):
    x = np.asarray(x, dtype=np.float32)
    z = np.asarray(z, dtype=np.float32)
    encoder_weight = np.asarray(encoder_weight, dtype=np.float32)
    encoder_bias = np.asarray(encoder_bias, dtype=np.float32)
    visible_bias = np.asarray(visible_bias, dtype=np.float32)
    pos_bias = np.asarray(pos_bias, dtype=np.float32)
    memory_weight = np.asarray(memory_weight, dtype=np.float32)
    W_Q = np.asarray(W_Q, dtype=np.float32)
    W_K = np.asarray(W_K, dtype=np.float32)

    bf = ml_dtypes.bfloat16
    # im2col staging: (b, c, (i pi), (j pj)) -> (b, (i j), (c pi pj));
    # visible/encoder/pos biases are pre-subtracted on the host so the
    # quad pass is a single Square+accum (no on-device subtract).
    xr_f = x.reshape(B, C, 8, P, 8, P).transpose(0, 2, 4, 1, 3, 5).reshape(
        B, NP, KCPP
    )
    vbp_f = (
        visible_bias.reshape(C, 8, P, 8, P).transpose(1, 3, 0, 2, 4).reshape(NP, KCPP)
    )
    # zb' absorbs the phi-bias/pos terms AND the Sum y.vbp correction
    # that the (x - vbp) substitution introduces in the enc product:
    # Sum_p y_p . vbp_p = Sum_p z_p . (Wenc vbp_p).
    zb_f = (
        encoder_bias[None, :] + pos_bias
        + vbp_f @ encoder_weight.reshape(D, KCPP).T
    )                                                                  # (NP, D)
    xr = np.ascontiguousarray(xr_f - vbp_f[None]).astype(bf)
    zr = z.astype(bf)
    zrs = (z - zb_f[None]).astype(bf)
    ztr = np.ascontiguousarray(zr.transpose(0, 2, 1))                 # (B, D, NP)
    mw_bf = memory_weight.astype(bf)                                   # (D, M)
    wqk = np.concatenate(
        [
            W_Q.transpose(2, 0, 1).reshape(D, NH * R),
            W_K.transpose(2, 0, 1).reshape(D, NH * R),
        ],
        axis=1,
    ).astype(bf)                                                       # (D, 512)
    wenc = encoder_weight.reshape(D, KCPP).astype(bf)                  # (D, 192)
    dmask = np.zeros((128, 4), dtype=bf)
    for hh in range(4):
        dmask[32 * hh : 32 * hh + 32, hh] = 1.0
    nmask = np.zeros((128, 2), dtype=bf)
    nmask[0:64, 0] = 1.0
    nmask[64:128, 1] = 1.0
    ident = np.eye(128, dtype=bf)

    host_corr = 0.5 * float(
        (vbp_f.astype(np.float64) ** 2).sum()
    ) + 0.5 * float((zb_f.astype(np.float64) ** 2).sum())

    if "nc" not in _CACHE:
        _CACHE["nc"] = _build_nc()
    nc = _CACHE["nc"]

    in_maps = []
    for c in range(N_CORES):
        sl = slice(c * BC, (c + 1) * BC)
        in_maps.append(
            {
                "x": xr[sl].reshape(BC // 2, 128, KCPP),
                "z": zrs[sl].reshape(BC // 2, 128, D),
                "ztr": ztr[sl],
                "mw": mw_bf,
                "wqk": wqk,
                "wenc": wenc,
                "dmask": dmask,
                "nmask": nmask,
                "ident": ident,
            }
        )
    _CACHE["last_in_maps"] = in_maps
    res = run_bass_kernel_spmd(nc, in_maps, list(range(N_CORES)))
    out = np.empty((B,), dtype=np.float32)
    for c in range(N_CORES):
        acc = res.results[c]["out"].reshape(128, NT, NB).astype(np.float64)
        s = np.stack([acc[:64].sum(0), acc[64:].sum(0)])   # (2, NT, NB)
        e = 0.5 * s[:, 0] - s[:, 1] - s[:, 2] - 4.0 * s[:, 3]  # (2, NB)
        out[c * BC : (c + 1) * BC] = e.T.reshape(BC)
    return (out - np.float32(host_corr)).astype(np.float32)

